# revision 1
# baseline (speedup 1.0000x reference)
"""Trainium2 Bass kernel for the unit-commitment custom loss.

Strategy (8 NeuronCores):
  - G (generator) dim sharded 8x500 for all (B,G,T)-shaped tensors and seg_prod.
  - B (scenario) dim sharded 8x2 for the P/S-shaped tensors and curtailment.
  - The device computes raw per-row (per-generator / per-profiled-unit /
    per-storage-unit) reduced quantities; the host folds the tiny per-row
    weights (min up/down masks, cost vectors) in float64 and sums.

Math for the min-up/down-time violations (all series are binary, so the
max() in the reference is a no-op and windowed sums become lag products):

  up(b,g)  = (U-1)*S0 - sum_{j=1..U-1} C_j      (restricted to t <= T-U)
  C_j      = sum_t sw_on[t]*s[t+j]              (computed on full range,
                                                 corner-corrected on host)
  dn(b,g)  = sum_{j=1..D-1} C'_j                (C'_j with sw_off)
  corrections use suffix sums of s over the last 7 steps (SC columns) and
  last-column sums of sw_on (SWT columns); early terms use prefix sums
  over the first 8 steps (PFB columns).
"""

import numpy as np

B, G, T, K, P, S = 16, 4000, 96, 4, 500, 200
M = 8            # cores
GC = G // M      # 500 generators per core
BS = B // M      # 2 scenarios per core (for P/S tensors)
GT = 4           # g partition tiles per core
GR = GC // GT    # 125 rows per tile
ST = 2           # s partition tiles
SR = S // ST     # 100 rows
NCOL = 64
VIOLATIONS_PENALTY = 1000.0
POWER_BALANCE_PENALTY = 5000.0

# column map (g rows)
C_ON0 = 0        # cols 0..6   : -C_j^on, j=1..7
C_OFF0 = 7       # cols 7..13  : -C_j^off
C_SWON = 14      # -sum sw_on
C_SCON0 = 15     # cols 15..20 : -SC_on, tau=2..7
C_SCOFF0 = 21    # cols 21..26 : -SC_off
C_SWT0 = 27      # cols 27..33 : -SWT, tau=1..7
C_PFB0 = 34      # cols 34..41 : PFB, r=1..8
C_SPK0 = 42      # cols 42..45 : sum seg_prod per k
C_TA = 46        # sum t*ln(p)   (thermal)
C_TB = 47        # sum t*ln1p(-p)
C_B = 48         # sum ln1p(-p)
C_PG = 49        # sum profiled_generation per p-row
# s rows (cols 52..59)
C_CR = 52
C_DR = 53
C_CHA = 54       # sum t*ln(p) charging
C_CHB = 55       # sum t*ln1p(-p)
C_CHC = 56       # sum ln1p(-p)
C_DSA = 57
C_DSB = 58
C_DSC = 59
C_CURT = 63      # rows 300..301

_NC = None


def _build_nc():
    import concourse.bacc as bacc
    import concourse.tile as tile
    import concourse.mybir as mybir

    dt = mybir.dt.float32
    alu = mybir.AluOpType
    AX = mybir.AxisListType
    LN = mybir.ActivationFunctionType.Ln

    nc = bacc.Bacc("TRN2", target_bir_lowering=False, debug=False, num_devices=M)

    s_ext = nc.dram_tensor("s_ext", [GC, B, T + 1], dt, kind="ExternalInput").ap()
    p_th = nc.dram_tensor("p_th", [GC, B, T], dt, kind="ExternalInput").ap()
    t_th = nc.dram_tensor("t_th", [GC, B, T], dt, kind="ExternalInput").ap()
    sp = nc.dram_tensor("sp", [GC, B, T, K], dt, kind="ExternalInput").ap()
    pg = nc.dram_tensor("pg", [P, BS, T], dt, kind="ExternalInput").ap()
    chp = nc.dram_tensor("chp", [S, BS, T], dt, kind="ExternalInput").ap()
    cht = nc.dram_tensor("cht", [S, BS, T], dt, kind="ExternalInput").ap()
    dsp = nc.dram_tensor("dsp", [S, BS, T], dt, kind="ExternalInput").ap()
    dst = nc.dram_tensor("dst", [S, BS, T], dt, kind="ExternalInput").ap()
    cr = nc.dram_tensor("cr", [S, BS, T], dt, kind="ExternalInput").ap()
    dr = nc.dram_tensor("dr", [S, BS, T], dt, kind="ExternalInput").ap()
    curt = nc.dram_tensor("curt", [BS, T], dt, kind="ExternalInput").ap()
    out = nc.dram_tensor("out", [512, NCOL], dt, kind="ExternalOutput").ap()

    with tile.TileContext(nc) as tc:
        with (
            tc.tile_pool(name="inp", bufs=2) as inp,
            tc.tile_pool(name="work", bufs=2) as work,
            tc.tile_pool(name="colp", bufs=2) as colp,
        ):
            for it in range(GT):
                r0 = it * GR
                sx_t = inp.tile([GR, B * (T + 1)], dt, tag="sx")
                nc.sync.dma_start(
                    sx_t[:], s_ext.rearrange("g b t -> g (b t)")[r0:r0 + GR, :])
                p_t = inp.tile([GR, B * T], dt, tag="p")
                nc.sync.dma_start(
                    p_t[:], p_th.rearrange("g b t -> g (b t)")[r0:r0 + GR, :])
                tt_t = inp.tile([GR, B * T], dt, tag="t")
                nc.sync.dma_start(
                    tt_t[:], t_th.rearrange("g b t -> g (b t)")[r0:r0 + GR, :])
                sp_t = inp.tile([GR, B * T * K], dt, tag="sp")
                nc.sync.dma_start(
                    sp_t[:], sp.rearrange("g b t k -> g (b t k)")[r0:r0 + GR, :])
                pg_t = inp.tile([GR, BS * T], dt, tag="pg")
                nc.sync.dma_start(
                    pg_t[:], pg.rearrange("p b t -> p (b t)")[r0:r0 + GR, :])

                cols = colp.tile([GR, 52], dt, tag="cols")
                nc.vector.memset(cols[:], 0.0)

                sv = sx_t[:].rearrange("g (b t) -> g b t", b=B)
                s = sv[:, :, 1:T + 1]
                pv = sv[:, :, 0:T]

                nswon = work.tile([GR, B * T], dt, tag="nswon")
                nswoff = work.tile([GR, B * T], dt, tag="nswoff")
                scr = work.tile([GR, B * T], dt, tag="scr")
                nwv = nswon[:].rearrange("g (b t) -> g b t", b=B)
                nfv = nswoff[:].rearrange("g (b t) -> g b t", b=B)
                scv = scr[:].rearrange("g (b t) -> g b t", b=B)

                # nsw_on = (prev - 1) * s ; accum -> -SWON
                nc.vector.scalar_tensor_tensor(
                    out=nwv, in0=pv, scalar=1.0, in1=s,
                    op0=alu.subtract, op1=alu.mult,
                    accum_out=cols[:, C_SWON:C_SWON + 1])
                # nsw_off = (s - 1) * prev
                nc.vector.scalar_tensor_tensor(
                    out=nfv, in0=s, scalar=1.0, in1=pv,
                    op0=alu.subtract, op1=alu.mult)

                # lag correlations, j = 1..7
                for j in range(1, 8):
                    nc.vector.scalar_tensor_tensor(
                        out=scv[:, :, 0:T - j],
                        in0=nwv[:, :, 0:T - j], scalar=1.0, in1=s[:, :, j:T],
                        op0=alu.mult, op1=alu.mult,
                        accum_out=cols[:, C_ON0 + j - 1:C_ON0 + j])
                    nc.vector.scalar_tensor_tensor(
                        out=scv[:, :, 0:T - j],
                        in0=nfv[:, :, 0:T - j], scalar=1.0, in1=s[:, :, j:T],
                        op0=alu.mult, op1=alu.mult,
                        accum_out=cols[:, C_OFF0 + j - 1:C_OFF0 + j])

                # corner suffix sums SS(tau) = sum_{u=1..tau-1} s[., T-u]
                ss = work.tile([GR, B], dt, tag="ss")
                scs = work.tile([GR, B], dt, tag="scs")
                nc.vector.tensor_copy(ss[:], s[:, :, T - 1])
                for tau in range(2, 8):
                    if tau > 2:
                        nc.vector.tensor_add(ss[:], ss[:], s[:, :, T + 1 - tau])
                    nc.vector.scalar_tensor_tensor(
                        out=scs[:], in0=nwv[:, :, T - tau], scalar=1.0, in1=ss[:],
                        op0=alu.mult, op1=alu.mult,
                        accum_out=cols[:, C_SCON0 + tau - 2:C_SCON0 + tau - 1])
                    nc.vector.scalar_tensor_tensor(
                        out=scs[:], in0=nfv[:, :, T - tau], scalar=1.0, in1=ss[:],
                        op0=alu.mult, op1=alu.mult,
                        accum_out=cols[:, C_SCOFF0 + tau - 2:C_SCOFF0 + tau - 1])

                # SWT(tau) = sum_b nsw_on[., T-tau], tau=1..7
                for tau in range(1, 8):
                    nc.vector.tensor_reduce(
                        cols[:, C_SWT0 + tau - 1:C_SWT0 + tau],
                        nwv[:, :, T - tau], axis=AX.X, op=alu.add)

                # prefix sums PF(r) = sum_{t<r} s, r=1..8
                pf = work.tile([GR, B], dt, tag="pf")
                nc.vector.tensor_copy(pf[:], s[:, :, 0])
                for r in range(1, 9):
                    if r > 1:
                        nc.vector.tensor_add(pf[:], pf[:], s[:, :, r - 1])
                    nc.vector.tensor_reduce(
                        cols[:, C_PFB0 + r - 1:C_PFB0 + r],
                        pf[:], axis=AX.X, op=alu.add)

                # seg_prod per-k row sums
                spv = sp_t[:].rearrange("g (b t k) -> g b t k", b=B, t=T)
                for k in range(K):
                    nc.vector.tensor_reduce(
                        cols[:, C_SPK0 + k:C_SPK0 + k + 1],
                        spv[:, :, :, k], axis=AX.XY, op=alu.add)

                # thermal BCE partials
                a_t = work.tile([GR, B * T], dt, tag="a")
                b_t = work.tile([GR, B * T], dt, tag="b")
                nc.scalar.activation(a_t[:], p_t[:], LN)
                nc.scalar.activation(b_t[:], p_t[:], LN, bias=1.0, scale=-1.0,
                                     accum_out=cols[:, C_B:C_B + 1])
                nc.vector.scalar_tensor_tensor(
                    out=scr[:], in0=tt_t[:], scalar=1.0, in1=a_t[:],
                    op0=alu.mult, op1=alu.mult,
                    accum_out=cols[:, C_TA:C_TA + 1])
                nc.vector.scalar_tensor_tensor(
                    out=scr[:], in0=tt_t[:], scalar=1.0, in1=b_t[:],
                    op0=alu.mult, op1=alu.mult,
                    accum_out=cols[:, C_TB:C_TB + 1])

                # profiled generation row sums
                nc.vector.tensor_reduce(
                    cols[:, C_PG:C_PG + 1],
                    pg_t[:].rearrange("p (b t) -> p b t", b=BS),
                    axis=AX.XY, op=alu.add)

                nc.sync.dma_start(out[r0:r0 + GR, 0:52], cols[:])

            # storage block: 2 tiles of 100 s-rows
            for st in range(ST):
                r0 = st * SR
                tiles = {}
                for name, src in (("chp", chp), ("cht", cht), ("dsp", dsp),
                                  ("dst", dst), ("cr", cr), ("dr", dr)):
                    tl = inp.tile([SR, BS * T], dt, tag="s_" + name)
                    nc.sync.dma_start(
                        tl[:], src.rearrange("s b t -> s (b t)")[r0:r0 + SR, :])
                    tiles[name] = tl
                scols = colp.tile([SR, 12], dt, tag="scols")
                nc.vector.memset(scols[:], 0.0)
                nc.vector.tensor_reduce(
                    scols[:, 0:1],
                    tiles["cr"][:].rearrange("s (b t) -> s b t", b=BS),
                    axis=AX.XY, op=alu.add)
                nc.vector.tensor_reduce(
                    scols[:, 1:2],
                    tiles["dr"][:].rearrange("s (b t) -> s b t", b=BS),
                    axis=AX.XY, op=alu.add)
                sa = work.tile([SR, BS * T], dt, tag="sa")
                sb = work.tile([SR, BS * T], dt, tag="sb")
                ssc = work.tile([SR, BS * T], dt, tag="ssc")
                for i, (pn, tn) in enumerate((("chp", "cht"), ("dsp", "dst"))):
                    c0 = 2 + 3 * i
                    nc.scalar.activation(sa[:], tiles[pn][:], LN)
                    nc.scalar.activation(sb[:], tiles[pn][:], LN, bias=1.0,
                                         scale=-1.0,
                                         accum_out=scols[:, c0 + 2:c0 + 3])
                    nc.vector.scalar_tensor_tensor(
                        out=ssc[:], in0=tiles[tn][:], scalar=1.0, in1=sa[:],
                        op0=alu.mult, op1=alu.mult,
                        accum_out=scols[:, c0:c0 + 1])
                    nc.vector.scalar_tensor_tensor(
                        out=ssc[:], in0=tiles[tn][:], scalar=1.0, in1=sb[:],
                        op0=alu.mult, op1=alu.mult,
                        accum_out=scols[:, c0 + 1:c0 + 2])
                nc.sync.dma_start(out[r0:r0 + SR, 52:64], scols[:])

            # curtailment
            ct = inp.tile([BS, T], dt, tag="curt")
            nc.sync.dma_start(ct[:], curt[:, :])
            ccol = colp.tile([BS, 1], dt, tag="ccol")
            nc.vector.tensor_reduce(ccol[:], ct[:], axis=AX.X, op=alu.add)
            nc.sync.dma_start(out[300:300 + BS, C_CURT:C_CURT + 1], ccol[:])

    nc.compile()
    return nc


def _get_nc():
    global _NC
    if _NC is None:
        _NC = _build_nc()
    return _NC


def _f32c(a):
    return np.ascontiguousarray(a, dtype=np.float32)


def _prep_in_maps(inputs):
    ic = np.asarray(inputs["initial_commitment"], dtype=np.float32)
    s_full = np.asarray(inputs["thermal_on_rounded"], dtype=np.float32)
    p_full = np.asarray(inputs["thermal_on"], dtype=np.float32)
    t_full = np.asarray(inputs["tgt_thermal_commitment"], dtype=np.float32)
    sp_full = np.asarray(inputs["seg_prod"], dtype=np.float32)
    pg_full = np.asarray(inputs["profiled_generation"], dtype=np.float32)
    chp_full = np.asarray(inputs["is_charging"], dtype=np.float32)
    cht_full = np.asarray(inputs["tgt_is_charging"], dtype=np.float32)
    dsp_full = np.asarray(inputs["is_discharging"], dtype=np.float32)
    dst_full = np.asarray(inputs["tgt_is_discharging"], dtype=np.float32)
    cr_full = np.asarray(inputs["charge_rate"], dtype=np.float32)
    dr_full = np.asarray(inputs["discharge_rate"], dtype=np.float32)
    curt_full = np.asarray(inputs["curtailment"], dtype=np.float32)

    in_maps = []
    for c in range(M):
        gsl = slice(GC * c, GC * (c + 1))
        bsl = slice(BS * c, BS * (c + 1))
        sx = np.empty((GC, B, T + 1), dtype=np.float32)
        sx[:, :, 0] = ic[:, gsl].T
        sx[:, :, 1:] = s_full[:, gsl].transpose(1, 0, 2)
        in_maps.append({
            "s_ext": sx,
            "p_th": _f32c(p_full[:, gsl].transpose(1, 0, 2)),
            "t_th": _f32c(t_full[:, gsl].transpose(1, 0, 2)),
            "sp": _f32c(sp_full[:, gsl].transpose(1, 0, 2, 3)),
            "pg": _f32c(pg_full[bsl].transpose(1, 0, 2)),
            "chp": _f32c(chp_full[bsl].transpose(1, 0, 2)),
            "cht": _f32c(cht_full[bsl].transpose(1, 0, 2)),
            "dsp": _f32c(dsp_full[bsl].transpose(1, 0, 2)),
            "dst": _f32c(dst_full[bsl].transpose(1, 0, 2)),
            "cr": _f32c(cr_full[bsl].transpose(1, 0, 2)),
            "dr": _f32c(dr_full[bsl].transpose(1, 0, 2)),
            "curt": _f32c(curt_full[bsl]),
        })
    return in_maps


def kernel(**inputs):
    from concourse.bass_utils import run_bass_kernel_spmd

    nc = _get_nc()
    in_maps = _prep_in_maps(inputs)
    res = run_bass_kernel_spmd(nc, in_maps, core_ids=list(range(M)))
    outs = [np.asarray(res.results[c]["out"], dtype=np.float64) for c in range(M)]
    return _combine(outs, inputs)


def _combine(outs, inputs):
    U_all = np.asarray(inputs["min_uptimes"]).astype(np.int64)
    D_all = np.asarray(inputs["min_downtimes"]).astype(np.int64)
    stat_all = np.asarray(inputs["initial_status"]).astype(np.int64)
    suc_all = np.asarray(inputs["start_up_costs"], dtype=np.float64)
    segc_all = np.asarray(inputs["segment_cost"], dtype=np.float64)[:, 0, :]
    puc = np.asarray(inputs["profiled_units_cost"], dtype=np.float64)
    ccost = np.asarray(inputs["charge_costs"], dtype=np.float64)
    dcost = np.asarray(inputs["discharge_costs"], dtype=np.float64)

    jj = np.arange(1, 8)[None, :]
    tt2 = np.arange(2, 8)[None, :]

    viol = 0.0
    ed = 0.0
    bce_th = 0.0
    bce_ch = 0.0
    bce_ds = 0.0
    curt_sum = 0.0

    for c in range(M):
        o = outs[c]
        R = o[0:GC, :]
        # g-block quantities (signs: device stored negatives for sw products)
        Con = -R[:, C_ON0:C_ON0 + 7]
        Coff = -R[:, C_OFF0:C_OFF0 + 7]
        SWON = -R[:, C_SWON]
        SCon = -R[:, C_SCON0:C_SCON0 + 6]
        SCoff = -R[:, C_SCOFF0:C_SCOFF0 + 6]
        SWT = -R[:, C_SWT0:C_SWT0 + 7]
        PFB = np.concatenate([np.zeros((GC, 1)), R[:, C_PFB0:C_PFB0 + 8]], axis=1)

        gsl = slice(GC * c, GC * (c + 1))
        U = U_all[gsl]
        D = D_all[gsl]
        stat = stat_all[gsl]

        S0 = SWON - (SWT * (jj < U[:, None])).sum(axis=1)
        up = ((U - 1) * S0).sum()
        up -= (Con * (jj < U[:, None])).sum()
        up += (SCon * (tt2 < U[:, None])).sum()
        dn = (Coff * (jj < D[:, None])).sum()
        dn -= (SCoff * (tt2 < D[:, None])).sum()
        rem_up = np.maximum(U - np.maximum(stat, 0), 0)
        rem_dn = np.maximum(D - np.maximum(-stat, 0), 0)
        g_idx = np.arange(GC)
        early = (B * rem_up - PFB[g_idx, rem_up]).sum() + PFB[g_idx, rem_dn].sum()
        viol += up + dn + early

        ed += (segc_all[gsl] * R[:, C_SPK0:C_SPK0 + K]).sum()
        ed += (suc_all[gsl] * SWON).sum()
        ed += (puc * R[:, C_PG]).sum()
        bce_th += R[:, C_TA].sum() + R[:, C_B].sum() - R[:, C_TB].sum()

        Srows = o[0:S, :]
        ed += (ccost * Srows[:, C_CR]).sum()
        ed += (dcost * Srows[:, C_DR]).sum()
        bce_ch += (Srows[:, C_CHA] + Srows[:, C_CHC] - Srows[:, C_CHB]).sum()
        bce_ds += (Srows[:, C_DSA] + Srows[:, C_DSC] - Srows[:, C_DSB]).sum()
        curt_sum += o[300:300 + BS, C_CURT].sum()

    n_th = float(B * G * T)
    n_s = float(B * S * T)
    sup = -(bce_th / n_th) - (bce_ch / n_s) - (bce_ds / n_s)
    total = ed + POWER_BALANCE_PENALTY * curt_sum + sup + VIOLATIONS_PENALTY * viol
    return np.float32(total)



# revision 5
# speedup vs baseline: 1.6136x; 1.6136x over previous
"""Trainium2 Bass kernel for the unit-commitment custom loss (v2).

Strategy (8 NeuronCores, SPMD single program):
  - Generator dim G sharded 8x500 for the (B,G,T) binary/prob tensors.
    Generators are permuted (lex-sorted desc by min-uptime U then
    min-downtime D, dealt into 32 cells of 125 rows, cells grouped into
    4 tile slots x 8 cores) so each 125-row tile only runs the lag
    passes j < max(U) (resp. max(D)) it actually needs.
  - All elementwise work is fp16 on the vector engine (2x DVE mode;
    binary data and counts <= 2048 are exact in fp16). Two parity
    copies of the commitment series (sE, sO = shift-by-1) keep every
    shifted operand 4-byte aligned so the 2x mode always engages.
  - seg_prod is staged fp8e4m3 as [B*T, G*K] and reduced on the idle
    tensor engine with ones-vector matmuls (PSUM accumulation), as are
    profiled_generation and charge/discharge rate row sums.
  - BCE terms use the binary-target select trick: q = 0.5+(2t-1)(p-0.5)
    so one Ln activation with accum_out per tensor pair replaces
    separate t*ln(p)/t*ln1p(-p) products.
  - Min-up/down-time violations: same lag-correlation algebra as the
    reference-equivalent decomposition (C_j columns + corner suffix
    corrections + prefix columns), with corner/prefix work batched into
    a few strided instructions. Host folds tiny per-g weights in f64.
"""

import numpy as np
import ml_dtypes

B, G, T, K, P, S = 16, 4000, 96, 4, 500, 200
M = 8            # cores
GC = G // M      # 500 generators per core
BS = B // M      # 2 scenarios per core (B-sharded tensors)
GT = 4           # g tile slots per core
GR = GC // GT    # 125 rows per tile
BT = B * T       # 1536
NCOL = 48
VIOLATIONS_PENALTY = 1000.0
POWER_BALANCE_PENALTY = 5000.0

# out_g column map (g rows, all accumulated with nsw = -sw sign)
C_ON0 = 0        # cols 0..6   : sum nsw_on * s[t+j], j=1..7  (= -C_on_j)
C_OFF0 = 7       # cols 7..13  : nsw_off version
C_SWON = 14      # sum nsw_on  (= -SWON)
C_SWT0 = 16      # cols 16..22 : i=0..6 -> tau=7-i : sum_b nsw_on[b, T-tau]
C_SCON0 = 23     # cols 23..29 : i -> tau=7-i : sum_b nsw_on[b,T-tau]*SS(tau)
C_SCOFF0 = 30    # cols 30..36
C_PFB0 = 37      # cols 37..45 : P_raw(r), r=0..8 (host: P(r)-P(0))
C_BCE = 46       # sum ln q  (thermal)

_NC_CACHE = {}


def _build_nc(n_on, n_off):
    import concourse.bacc as bacc
    import concourse.tile as tile
    import concourse.mybir as mybir

    f32 = mybir.dt.float32
    f16 = mybir.dt.float16
    f8 = mybir.dt.float8e4
    alu = mybir.AluOpType
    AX = mybir.AxisListType
    LN = mybir.ActivationFunctionType.Ln
    CP = mybir.ActivationFunctionType.Copy

    nc = bacc.Bacc("TRN2", target_bir_lowering=False, debug=False, num_devices=M)

    # const AP for the Ln(v + 0.5) bias used by the BCE select trick
    _half = nc.alloc_sbuf_tensor("const-half", [128, 1], f32)
    nc.gpsimd.memset(_half.ap(), 0.5)
    nc.const_aps.aps[(f32, 0.5)] = _half.ap()
    nc.all_engine_barrier()

    sE = nc.dram_tensor("sE", [GC, BT], f16, kind="ExternalInput").ap()
    sO = nc.dram_tensor("sO", [GC, BT], f16, kind="ExternalInput").ap()
    pE = nc.dram_tensor("pE", [GC, BT], f16, kind="ExternalInput").ap()
    pp = nc.dram_tensor("pp", [GC, BT], f16, kind="ExternalInput").ap()
    tt = nc.dram_tensor("tt", [GC, BT], f16, kind="ExternalInput").ap()
    sp8 = nc.dram_tensor("sp8", [BT, GC * K], f8, kind="ExternalInput").ap()
    pg16 = nc.dram_tensor("pg16", [T, BS * P], f16, kind="ExternalInput").ap()
    st6 = nc.dram_tensor("st6", [T, 6 * S * BS + BS], f16, kind="ExternalInput").ap()
    ones16 = nc.dram_tensor("ones16", [128, 2], f16, kind="ExternalInput").ap()
    ones8 = nc.dram_tensor("ones8", [128, 2], f8, kind="ExternalInput").ap()

    out_g = nc.dram_tensor("out_g", [GC, NCOL], f32, kind="ExternalOutput").ap()
    out_s = nc.dram_tensor("out_s", [T, 8], f32, kind="ExternalOutput").ap()
    out_pe = nc.dram_tensor("out_pe", [1, 3300], f32, kind="ExternalOutput").ap()

    with tile.TileContext(nc) as tc:
        with (
            tc.tile_pool(name="inp", bufs=2) as inp,
            tc.tile_pool(name="work", bufs=2) as work,
            tc.tile_pool(name="colp", bufs=2) as colp,
            tc.tile_pool(name="cst", bufs=1) as cst,
            tc.tile_pool(name="ps", bufs=1, space="PSUM") as psp,
        ):
            o16 = cst.tile([128, 2], f16, tag="o16")
            nc.sync.dma_start(o16[:], ones16[:, :])
            o8 = cst.tile([128, 2], f8, tag="o8")
            nc.sync.dma_start(o8[:], ones8[:, :])

            # ---- seg_prod reduction on the tensor engine ----
            ps_seg = [psp.tile([1, 500], f32, tag=f"seg{q}", name=f"ps_seg{q}")
                      for q in range(4)]
            for ch in range(12):
                t8 = inp.tile([128, GC * K], f8, tag="sp")
                nc.sync.dma_start(t8[:], sp8[128 * ch:128 * (ch + 1), :])
                for q in range(4):
                    nc.tensor.matmul(
                        ps_seg[q][:], o8[:, 0:1], t8[:, 500 * q:500 * (q + 1)],
                        start=(ch == 0), stop=(ch == 11))

            # ---- profiled_generation row sums ----
            ps_pg = psp.tile([1, P], f32, tag="pg")
            pgt = inp.tile([T, BS * P], f16, tag="pg")
            nc.sync.dma_start(pgt[:], pg16[:, :])
            for b in range(2):
                nc.tensor.matmul(
                    ps_pg[:], o16[0:96, 0:1], pgt[:, P * b:P * (b + 1)],
                    start=(b == 0), stop=(b == 1))

            # ---- storage block ----
            SB = S * BS  # 400
            stt = inp.tile([T, 6 * SB + BS], f16, tag="st")
            nc.sync.dma_start(stt[:], st6[:, :])
            ps_cr = psp.tile([1, SB], f32, tag="cr")
            nc.tensor.matmul(ps_cr[:], o16[0:96, 0:1], stt[:, 4 * SB:5 * SB],
                             start=True, stop=True)
            ps_dr = psp.tile([1, SB], f32, tag="dr")
            nc.tensor.matmul(ps_dr[:], o16[0:96, 0:1], stt[:, 5 * SB:6 * SB],
                             start=True, stop=True)

            scols = colp.tile([T, 8], f32, tag="scols")
            nc.vector.memset(scols[:], 0.0)
            su = work.tile([T, SB], f16, tag="su")
            sv = work.tile([T, SB], f16, tag="sv")
            sq = work.tile([T, SB], f16, tag="sq")
            for i in range(2):
                pr = stt[:, (2 * i) * SB:(2 * i + 1) * SB]
                tg = stt[:, (2 * i + 1) * SB:(2 * i + 2) * SB]
                nc.scalar.activation(su[:], tg, CP, bias=-1.0, scale=2.0)
                nc.vector.scalar_tensor_tensor(
                    out=sv[:], in0=pr, scalar=0.5, in1=su[:],
                    op0=alu.subtract, op1=alu.mult)
                nc.scalar.activation(sq[:], sv[:], LN, bias=0.5, scale=1.0,
                                     accum_out=scols[:, i:i + 1])
            nc.vector.tensor_reduce(
                scols[:, 2:3], stt[:, 6 * SB:6 * SB + BS], axis=AX.X, op=alu.add)
            nc.sync.dma_start(out_s[:, :], scols[:])

            # ---- assemble PE results ----
            peout = colp.tile([1, 3300], f32, tag="peout")
            for q in range(4):
                nc.scalar.activation(
                    peout[:, 500 * q:500 * (q + 1)], ps_seg[q][:], CP)
            nc.scalar.activation(peout[:, 2000:2500], ps_pg[:], CP)
            nc.scalar.activation(peout[:, 2500:2900], ps_cr[:], CP)
            nc.scalar.activation(peout[:, 2900:3300], ps_dr[:], CP)
            nc.sync.dma_start(out_pe[:, :], peout[:])

            # ---- generator tiles ----
            for it in range(GT):
                r0 = it * GR
                se = inp.tile([GR, BT], f16, tag="se")
                nc.sync.dma_start(se[:], sE[r0:r0 + GR, :])
                so = inp.tile([GR, BT], f16, tag="so")
                nc.sync.dma_start(so[:], sO[r0:r0 + GR, :])
                pe = inp.tile([GR, BT], f16, tag="pe")
                nc.sync.dma_start(pe[:], pE[r0:r0 + GR, :])
                p_t = inp.tile([GR, BT], f16, tag="pp")
                nc.sync.dma_start(p_t[:], pp[r0:r0 + GR, :])
                t_t = inp.tile([GR, BT], f16, tag="tt")
                nc.sync.dma_start(t_t[:], tt[r0:r0 + GR, :])

                cols = colp.tile([GR, NCOL], f32, tag="cols")
                nc.vector.memset(cols[:], 0.0)

                sev = se[:].rearrange("g (b t) -> g b t", b=B)
                sov = so[:].rearrange("g (b t) -> g b t", b=B)

                non = work.tile([GR, BT], f16, tag="non")
                nof = work.tile([GR, BT], f16, tag="nof")
                scr = work.tile([GR, BT], f16, tag="scr")
                nv = non[:].rearrange("g (b t) -> g b t", b=B)
                fv = nof[:].rearrange("g (b t) -> g b t", b=B)
                cv = scr[:].rearrange("g (b t) -> g b t", b=B)

                # nsw_on = (prev - 1) * s ; nsw_off = (s - 1) * prev
                nc.vector.scalar_tensor_tensor(
                    out=non[:], in0=pe[:], scalar=1.0, in1=se[:],
                    op0=alu.subtract, op1=alu.mult,
                    accum_out=cols[:, C_SWON:C_SWON + 1])
                nc.vector.scalar_tensor_tensor(
                    out=nof[:], in0=se[:], scalar=1.0, in1=pe[:],
                    op0=alu.subtract, op1=alu.mult)

                # lag correlations (only as many shifts as this tile needs)
                for j in range(1, n_on[it] + 1):
                    src = sev[:, :, j:T] if j % 2 == 0 else sov[:, :, j - 1:T - 1]
                    nc.vector.scalar_tensor_tensor(
                        out=cv[:, :, 0:T - j], in0=nv[:, :, 0:T - j],
                        scalar=1.0, in1=src, op0=alu.mult, op1=alu.mult,
                        accum_out=cols[:, C_ON0 + j - 1:C_ON0 + j])
                for j in range(1, n_off[it] + 1):
                    src = sev[:, :, j:T] if j % 2 == 0 else sov[:, :, j - 1:T - 1]
                    nc.vector.scalar_tensor_tensor(
                        out=cv[:, :, 0:T - j], in0=fv[:, :, 0:T - j],
                        scalar=1.0, in1=src, op0=alu.mult, op1=alu.mult,
                        accum_out=cols[:, C_OFF0 + j - 1:C_OFF0 + j])

                # corner suffix sums: ss2[:, b, i] = SS(tau=7-i)
                # SS(tau) = sum_{u=1..tau-1} s[b, T-u]
                sst = work.tile([GR, B * 7], f16, tag="sst")
                ssv = sst[:].rearrange("g (b i) -> g b i", b=B)
                nc.vector.memset(sst[:], 0.0)
                for tau in range(2, 8):
                    i = 7 - tau
                    nc.vector.tensor_add(
                        ssv[:, :, i], ssv[:, :, i + 1], sev[:, :, T + 1 - tau])

                # last-7 views (i=0..6 -> t = T-7+i = 89+i -> tau = 7-i)
                non7 = nv[:, :, T - 7:T]
                nof7 = fv[:, :, T - 7:T]
                prd = work.tile([GR, B * 7], f16, tag="prd")
                pv = prd[:].rearrange("g (b i) -> g b i", b=B)
                pvr = prd[:].rearrange("g (b i) -> g i b", b=B)

                nc.vector.tensor_mul(pv[:, :, :], non7, ssv[:, :, :])
                nc.vector.tensor_reduce(
                    cols[:, C_SCON0:C_SCON0 + 7], pvr, axis=AX.X, op=alu.add)
                nc.vector.tensor_mul(pv[:, :, :], nof7, ssv[:, :, :])
                nc.vector.tensor_reduce(
                    cols[:, C_SCOFF0:C_SCOFF0 + 7], pvr, axis=AX.X, op=alu.add)

                # SWT: sum_b nsw_on[b, 89+i]
                non7r = non[:].rearrange("g (b t) -> g t b", b=B)[:, T - 7:T, :]
                nc.vector.tensor_reduce(
                    cols[:, C_SWT0:C_SWT0 + 7], non7r, axis=AX.X, op=alu.add)

                # prefix columns: P_raw(r) = sum_b csx[8b + r], r=0..8
                c8 = work.tile([GR, 128], f16, tag="c8")
                nc.vector.tensor_copy(
                    c8[:].rearrange("g (b r) -> g b r", b=B), sev[:, :, 0:8])
                csx = work.tile([GR, 136], f16, tag="csx")
                nc.vector.memset(csx[:, 0:1], 0.0)
                nc.vector.tensor_tensor_scan(
                    out=csx[:, 1:129], data0=c8[:], data1=c8[:],
                    initial=0.0, op0=alu.add, op1=alu.bypass)
                # P_raw(r) = sum_b csx[8b + r]: r=0..7 in one strided reduce,
                # r=8 (cols 8b+8) separately.
                nc.vector.tensor_reduce(
                    cols[:, C_PFB0:C_PFB0 + 8],
                    csx[:, 0:128].rearrange("g (b r) -> g r b", r=8),
                    axis=AX.X, op=alu.add)
                nc.vector.tensor_reduce(
                    cols[:, C_PFB0 + 8:C_PFB0 + 9],
                    csx[:, 8:136].rearrange("g (b r) -> g b r", r=8)[:, :, 0],
                    axis=AX.X, op=alu.add)

                # thermal BCE: q = 0.5 + (2t-1)(p-0.5)
                ub = work.tile([GR, BT], f16, tag="ub")
                nc.scalar.activation(ub[:], t_t[:], CP, bias=-1.0, scale=2.0)
                nc.vector.scalar_tensor_tensor(
                    out=scr[:], in0=p_t[:], scalar=0.5, in1=ub[:],
                    op0=alu.subtract, op1=alu.mult)
                nc.scalar.activation(ub[:], scr[:], LN, bias=0.5, scale=1.0,
                                     accum_out=cols[:, C_BCE:C_BCE + 1])

                nc.sync.dma_start(out_g[r0:r0 + GR, :], cols[:])

    nc.compile()
    return nc


def _get_nc(n_on, n_off):
    key = (tuple(n_on), tuple(n_off))
    if key not in _NC_CACHE:
        _NC_CACHE[key] = _build_nc(n_on, n_off)
    return _NC_CACHE[key]


def _plan(U, D):
    """Permute generators to minimize lag passes.

    Lex-sort by (U desc, D desc), cut into 32 cells of GR rows, sort
    cells by (maxU desc, maxD desc), deal cell k to (tile k//M, core k%M).
    Returns (rows[core][tile] -> array of g ids, n_on[GT], n_off[GT]).
    """
    order = np.lexsort((-D, -U))
    cells = [order[GR * i:GR * (i + 1)] for i in range(M * GT)]
    ckey = [(-U[c].max(), -D[c].max()) for c in cells]
    csort = sorted(range(M * GT), key=lambda i: ckey[i])
    rows = [[None] * GT for _ in range(M)]
    n_on = [0] * GT
    n_off = [0] * GT
    for k, ci in enumerate(csort):
        t, c = k // M, k % M
        rows[c][t] = cells[ci]
        n_on[t] = max(n_on[t], int(U[cells[ci]].max()) - 1)
        n_off[t] = max(n_off[t], int(D[cells[ci]].max()) - 1)
    return rows, n_on, n_off


def _f16(a):
    return np.ascontiguousarray(a, dtype=np.float16)


def _prep_in_maps(inputs, rows):
    ic = np.asarray(inputs["initial_commitment"], dtype=np.float32)      # (B,G)
    s_full = np.asarray(inputs["thermal_on_rounded"], dtype=np.float32)  # (B,G,T)
    p_full = np.asarray(inputs["thermal_on"], dtype=np.float32)
    t_full = np.asarray(inputs["tgt_thermal_commitment"], dtype=np.float32)
    sp_full = np.asarray(inputs["seg_prod"], dtype=np.float32)           # (B,G,T,K)
    pg_full = np.asarray(inputs["profiled_generation"], dtype=np.float32)
    chp = np.asarray(inputs["is_charging"], dtype=np.float32)            # (B,S,T)
    cht = np.asarray(inputs["tgt_is_charging"], dtype=np.float32)
    dsp = np.asarray(inputs["is_discharging"], dtype=np.float32)
    dst = np.asarray(inputs["tgt_is_discharging"], dtype=np.float32)
    cr = np.asarray(inputs["charge_rate"], dtype=np.float32)
    dr = np.asarray(inputs["discharge_rate"], dtype=np.float32)
    curt = np.asarray(inputs["curtailment"], dtype=np.float32)           # (B,T)

    ones16 = np.ones((128, 2), dtype=np.float16)
    ones8 = np.ones((128, 2), dtype=ml_dtypes.float8_e4m3)

    in_maps = []
    for c in range(M):
        gids = np.concatenate(rows[c])                       # (500,)
        s = s_full[:, gids, :].transpose(1, 0, 2)            # (500,B,T)
        sEc = _f16(s).reshape(GC, BT)
        sOc = np.zeros((GC, B, T), dtype=np.float16)
        sOc[:, :, :T - 1] = s[:, :, 1:]
        pEc = np.empty((GC, B, T), dtype=np.float16)
        pEc[:, :, 0] = ic[:, gids].T
        pEc[:, :, 1:] = s[:, :, :T - 1]

        gsl = slice(GC * c, GC * (c + 1))
        bsl = slice(BS * c, BS * (c + 1))
        sp8c = np.ascontiguousarray(
            sp_full[:, gsl].transpose(0, 2, 1, 3).reshape(BT, GC * K)
        ).astype(ml_dtypes.float8_e4m3)
        # (BS,P,T) -> (T, BS*P), col = b*P + p
        pgc = _f16(pg_full[bsl].transpose(2, 0, 1).reshape(T, BS * P))

        def sb(x):  # (BS,S,T) -> (T, S*BS) with col = s*BS + b
            return x[bsl].transpose(2, 1, 0).reshape(T, S * BS)

        st6c = np.concatenate(
            [sb(chp), sb(cht), sb(dsp), sb(dst), sb(cr), sb(dr),
             curt[bsl].T], axis=1)

        in_maps.append({
            "sE": sEc,
            "sO": sOc.reshape(GC, BT),
            "pE": pEc.reshape(GC, BT),
            "pp": _f16(p_full[:, gids].transpose(1, 0, 2)).reshape(GC, BT),
            "tt": _f16(t_full[:, gids].transpose(1, 0, 2)).reshape(GC, BT),
            "sp8": sp8c,
            "pg16": pgc,
            "st6": _f16(st6c),
            "ones16": ones16,
            "ones8": ones8,
        })
    return in_maps


def kernel(**inputs):
    from concourse.bass_utils import run_bass_kernel_spmd

    U_all = np.maximum(np.asarray(inputs["min_uptimes"]).astype(np.int64), 0)
    D_all = np.maximum(np.asarray(inputs["min_downtimes"]).astype(np.int64), 0)
    rows, n_on, n_off = _plan(U_all, D_all)
    nc = _get_nc(n_on, n_off)
    in_maps = _prep_in_maps(inputs, rows)
    res = run_bass_kernel_spmd(nc, in_maps, core_ids=list(range(M)))
    return _combine(res.results, inputs, rows)


def _combine(results, inputs, rows):
    U_all = np.asarray(inputs["min_uptimes"]).astype(np.int64)
    D_all = np.asarray(inputs["min_downtimes"]).astype(np.int64)
    stat_all = np.asarray(inputs["initial_status"]).astype(np.int64)
    suc_all = np.asarray(inputs["start_up_costs"], dtype=np.float64)
    segc_all = np.asarray(inputs["segment_cost"], dtype=np.float64)[:, 0, :]
    puc = np.asarray(inputs["profiled_units_cost"], dtype=np.float64)
    ccost = np.asarray(inputs["charge_costs"], dtype=np.float64)
    dcost = np.asarray(inputs["discharge_costs"], dtype=np.float64)

    jj = np.arange(1, 8)[None, :]          # lag index
    tt_i = 7 - np.arange(7)[None, :]       # column i -> tau = 7-i

    viol = 0.0
    ed = 0.0
    bce_th = 0.0
    bce_ch = 0.0
    bce_ds = 0.0
    curt_sum = 0.0

    for c in range(M):
        o = np.asarray(results[c]["out_g"], dtype=np.float64)
        ope = np.asarray(results[c]["out_pe"], dtype=np.float64)[0]
        osr = np.asarray(results[c]["out_s"], dtype=np.float64)

        gids = np.concatenate(rows[c])
        U = U_all[gids]
        D = D_all[gids]
        stat = stat_all[gids]

        Con = -o[:, C_ON0:C_ON0 + 7]
        Coff = -o[:, C_OFF0:C_OFF0 + 7]
        SWON = -o[:, C_SWON]
        SWT = -o[:, C_SWT0:C_SWT0 + 7]      # col i -> tau=7-i
        SCon = -o[:, C_SCON0:C_SCON0 + 7]
        SCoff = -o[:, C_SCOFF0:C_SCOFF0 + 7]
        Praw = o[:, C_PFB0:C_PFB0 + 9]
        Pf = Praw - Praw[:, 0:1]            # P(r), r=0..8

        mU = (tt_i < U[:, None])
        S0 = SWON - (SWT * mU).sum(axis=1)
        up = ((U - 1) * S0).sum()
        up -= (Con * (jj < U[:, None])).sum()
        up += (SCon * ((tt_i >= 2) & (tt_i < U[:, None]))).sum()
        dn = (Coff * (jj < D[:, None])).sum()
        dn -= (SCoff * ((tt_i >= 2) & (tt_i < D[:, None]))).sum()
        rem_up = np.maximum(U - np.maximum(stat, 0), 0)
        rem_dn = np.maximum(D - np.maximum(-stat, 0), 0)
        g_idx = np.arange(GC)
        early = (B * rem_up - Pf[g_idx, rem_up]).sum() + Pf[g_idx, rem_dn].sum()
        viol += up + dn + early

        ed += (suc_all[gids] * SWON).sum()
        bce_th += o[:, C_BCE].sum()

        gsl = slice(GC * c, GC * (c + 1))
        ed += (segc_all[gsl].reshape(-1) * ope[0:2000]).sum()
        ed += (puc * ope[2000:2500]).sum()
        ed += (ccost * ope[2500:2900].reshape(S, BS).sum(axis=1)).sum()
        ed += (dcost * ope[2900:3300].reshape(S, BS).sum(axis=1)).sum()

        bce_ch += osr[:, 0].sum()
        bce_ds += osr[:, 1].sum()
        curt_sum += osr[:, 2].sum()

    n_th = float(B * G * T)
    n_s = float(B * S * T)
    sup = -(bce_th / n_th) - (bce_ch / n_s) - (bce_ds / n_s)
    total = (ed + POWER_BALANCE_PENALTY * curt_sum + sup
             + VIOLATIONS_PENALTY * viol)
    return np.float32(total)
